# revision 7
# baseline (speedup 1.0000x reference)
"""Trainium2 Bass kernel for nn_ActorNetSpiking (4-layer LIF SNN, T=50).

Contract: kernel(**inputs) takes FULL unsharded inputs (x:[4096,512,50] f32,
W1..W4/b1..b4, batch_size) and returns the FULL [4096,2] f32 output.

Strategy: pure data parallel over 8 NeuronCores (batch 4096 -> 512/core).
Host pre-transposes each core's x shard to [T, S, B] so the kernel can DMA
[S=512, B=512] slices with S on partitions (contraction dim for the L1
matmul).  All activations live transposed on-chip: [feature, batch].

Per timestep t (fully unrolled, Tile schedules everything):
  PE : cur1 = W1 @ x_t   (4 K-chunks x 2 M-chunks, + rank-1 bias)
       cur2 = W2 @ spk1, cur3 = W3 @ spk2, cur4 = W4 @ spk3
  DVE: per layer  syn' = alpha*syn + cur ; u = beta*mem + syn' ;
       mem' = -spk_prev + u ; spk' = (mem' > 1)
       acc += spk4
Output: raw spike counts acc [2, 512] per core; host divides by T and
re-assembles [4096, 2].
"""

import sys

sys.path.insert(0, "/opt/trn_rl_repo")

from contextlib import ExitStack

import numpy as np

import concourse.bass as bass
import concourse.bacc as bacc
import concourse.tile as tile
from concourse import mybir

F32 = mybir.dt.float32
ALU = mybir.AluOpType

ALPHA = 0.9
BETA = 0.85
THR = 1.0

N_CORES = 8
B_FULL = 4096
S = 512
H = 256
A = 2
T_FULL = 50
B = B_FULL // N_CORES  # 512 per core


def build_nc(T=T_FULL, vec_engines=("vector",)):
    """Build the single-core Bass program (SPMD: same program on all cores)."""
    nc = bacc.Bacc(None, target_bir_lowering=False)

    xt = nc.declare_dram_parameter("xt", [T, S, B], F32, isOutput=False)
    w1t = nc.declare_dram_parameter("w1t", [S, H], F32, isOutput=False)
    w2t = nc.declare_dram_parameter("w2t", [H, H], F32, isOutput=False)
    w3t = nc.declare_dram_parameter("w3t", [H, H], F32, isOutput=False)
    w4t = nc.declare_dram_parameter("w4t", [H, A], F32, isOutput=False)
    b1 = nc.declare_dram_parameter("b1", [1, H], F32, isOutput=False)
    b2 = nc.declare_dram_parameter("b2", [1, H], F32, isOutput=False)
    b3 = nc.declare_dram_parameter("b3", [1, H], F32, isOutput=False)
    b4 = nc.declare_dram_parameter("b4", [1, A], F32, isOutput=False)
    out = nc.declare_dram_parameter("out", [A, B], F32, isOutput=True)

    KC1 = S // 128  # 4 k-chunks for layer 1
    KC = H // 128  # 2 k-chunks for layers 2-4
    MC = H // 128  # 2 m-chunks for layers 1-3

    # round-robin engine choice for the elementwise ops
    engines = [getattr(nc, e) for e in vec_engines]
    _eng_i = [0]

    def veng():
        e = engines[_eng_i[0] % len(engines)]
        _eng_i[0] += 1
        return e

    with tile.TileContext(nc) as tc, ExitStack() as ctx:
        wp = ctx.enter_context(tc.tile_pool(name="weights", bufs=1))
        xp = ctx.enter_context(tc.tile_pool(name="x", bufs=3))
        sp = ctx.enter_context(tc.tile_pool(name="state", bufs=2))
        tp = ctx.enter_context(tc.tile_pool(name="tmp", bufs=2))
        pp = ctx.enter_context(tc.tile_pool(name="psum", bufs=1, space="PSUM"))

        # --- load weights / biases ---
        w1 = []
        for k in range(KC1):
            wt = wp.tile([128, H], F32, tag=f"w1_{k}")
            nc.sync.dma_start(wt[:], w1t[k * 128 : (k + 1) * 128, :])
            w1.append(wt)
        w23 = {}
        for name, dram in (("w2", w2t), ("w3", w3t)):
            lst = []
            for k in range(KC):
                wt = wp.tile([128, H], F32, tag=f"{name}_{k}")
                nc.sync.dma_start(wt[:], dram[k * 128 : (k + 1) * 128, :])
                lst.append(wt)
            w23[name] = lst
        w4 = []
        for k in range(KC):
            wt = wp.tile([128, A], F32, tag=f"w4_{k}")
            nc.sync.dma_start(wt[:], w4t[k * 128 : (k + 1) * 128, :])
            w4.append(wt)
        bt = {}
        for name, dram, n in (("b1", b1, H), ("b2", b2, H), ("b3", b3, H), ("b4", b4, A)):
            t_ = wp.tile([1, n], F32, tag=name)
            nc.sync.dma_start(t_[:], dram[:])
            bt[name] = t_
        ones = wp.tile([1, B], F32, tag="ones")
        nc.vector.memset(ones[:], 1.0)

        # --- initial states (zeros) ---
        state = {}
        for L, nchunk, pdim in ((1, MC, 128), (2, MC, 128), (3, MC, 128), (4, 1, A)):
            for kind in ("syn", "mem", "spk"):
                for m in range(nchunk):
                    t_ = sp.tile([pdim, B], F32, tag=f"{kind}{L}_{m}")
                    nc.vector.memset(t_[:], 0.0)
                    state[(kind, L, m)] = t_
        acc = sp.tile([A, B], F32, tag="acc")
        nc.vector.memset(acc[:], 0.0)

        def lif_update(L, m, pdim, cur_psum):
            """syn' = a*syn+cur; u = b*mem+syn'; mem' = -spk+u; spk' = mem'>1"""
            syn_o, mem_o, spk_o = (
                state[("syn", L, m)],
                state[("mem", L, m)],
                state[("spk", L, m)],
            )
            syn_n = sp.tile([pdim, B], F32, tag=f"syn{L}_{m}")
            veng().scalar_tensor_tensor(
                syn_n[:], syn_o[:], ALPHA, cur_psum[:], op0=ALU.mult, op1=ALU.add
            )
            u = tp.tile([pdim, B], F32, tag=f"u{L}_{m}")
            veng().scalar_tensor_tensor(
                u[:], mem_o[:], BETA, syn_n[:], op0=ALU.mult, op1=ALU.add
            )
            mem_n = sp.tile([pdim, B], F32, tag=f"mem{L}_{m}")
            veng().scalar_tensor_tensor(
                mem_n[:], spk_o[:], -THR, u[:], op0=ALU.mult, op1=ALU.add
            )
            spk_n = sp.tile([pdim, B], F32, tag=f"spk{L}_{m}")
            veng().tensor_scalar(spk_n[:], mem_n[:], THR, None, op0=ALU.is_gt)
            state[("syn", L, m)] = syn_n
            state[("mem", L, m)] = mem_n
            state[("spk", L, m)] = spk_n

        for t in range(T):
            # load x_t: [S, B] as 4 partition chunks
            xtiles = []
            for k in range(KC1):
                xt_k = xp.tile([128, B], F32, tag=f"x_{k}")
                nc.sync.dma_start(xt_k[:], xt[t, k * 128 : (k + 1) * 128, :])
                xtiles.append(xt_k)

            # ---- layer 1 ----
            ps1 = []
            for m in range(MC):
                ps = pp.tile([128, B], F32, tag=f"ps1_{m}")
                for k in range(KC1):
                    nc.tensor.matmul(
                        ps[:],
                        w1[k][:, m * 128 : (m + 1) * 128],
                        xtiles[k][:],
                        start=(k == 0),
                        stop=False,
                    )
                nc.tensor.matmul(
                    ps[:],
                    bt["b1"][:, m * 128 : (m + 1) * 128],
                    ones[:],
                    start=False,
                    stop=True,
                )
                ps1.append(ps)
            for m in range(MC):
                lif_update(1, m, 128, ps1[m])

            # ---- layers 2, 3 ----
            for L, wname, bname in ((2, "w2", "b2"), (3, "w3", "b3")):
                psl = []
                for m in range(MC):
                    ps = pp.tile([128, B], F32, tag=f"ps{L}_{m}")
                    for k in range(KC):
                        nc.tensor.matmul(
                            ps[:],
                            w23[wname][k][:, m * 128 : (m + 1) * 128],
                            state[("spk", L - 1, k)][:],
                            start=(k == 0),
                            stop=False,
                        )
                    nc.tensor.matmul(
                        ps[:],
                        bt[bname][:, m * 128 : (m + 1) * 128],
                        ones[:],
                        start=False,
                        stop=True,
                    )
                    psl.append(ps)
                for m in range(MC):
                    lif_update(L, m, 128, psl[m])

            # ---- layer 4 ----
            ps4 = pp.tile([A, B], F32, tag="ps4")
            for k in range(KC):
                nc.tensor.matmul(
                    ps4[:],
                    w4[k][:],
                    state[("spk", 3, k)][:],
                    start=(k == 0),
                    stop=False,
                )
            nc.tensor.matmul(ps4[:], bt["b4"][:], ones[:], start=False, stop=True)
            lif_update(4, 0, A, ps4)

            acc_n = sp.tile([A, B], F32, tag="acc")
            veng().tensor_tensor(
                acc_n[:], acc[:], state[("spk", 4, 0)][:], op=ALU.add
            )
            acc = acc_n

        res = tp.tile([A, B], F32, tag="res")
        nc.vector.tensor_copy(res[:], acc[:])
        nc.sync.dma_start(out[:], res[:])

    nc.finalize()
    return nc


def make_in_maps(x, W1, b1, W2, b2, W3, b3, W4, b4, T=T_FULL):
    """Shard + transpose full inputs into per-core input maps."""
    common = {
        "w1t": np.ascontiguousarray(W1.T),
        "w2t": np.ascontiguousarray(W2.T),
        "w3t": np.ascontiguousarray(W3.T),
        "w4t": np.ascontiguousarray(W4.T),
        "b1": np.ascontiguousarray(b1.reshape(1, -1)),
        "b2": np.ascontiguousarray(b2.reshape(1, -1)),
        "b3": np.ascontiguousarray(b3.reshape(1, -1)),
        "b4": np.ascontiguousarray(b4.reshape(1, -1)),
    }
    in_maps = []
    for c in range(N_CORES):
        xs = x[c * B : (c + 1) * B, :, :T]  # [B, S, T]
        xtc = np.ascontiguousarray(xs.transpose(2, 1, 0))  # [T, S, B]
        m = dict(common)
        m["xt"] = xtc
        in_maps.append(m)
    return in_maps


def assemble_output(results, T=T_FULL):
    """results: list of per-core dicts with 'out' [A, B] raw spike counts."""
    outs = []
    for c in range(N_CORES):
        acc = results[c]["out"]  # [A, B]
        outs.append(acc.T)  # [B, A]
    full = np.concatenate(outs, axis=0)  # [4096, A]
    return (full / np.float32(T)).astype(np.float32)


_NC_CACHE = {}


def kernel(x, W1, b1, W2, b2, W3, b3, W4, b4, batch_size=None, **_):
    x = np.asarray(x, np.float32)
    args = [np.asarray(a, np.float32) for a in (W1, b1, W2, b2, W3, b3, W4, b4)]
    from concourse.bass_utils import run_bass_kernel_spmd

    key = "main"
    if key not in _NC_CACHE:
        _NC_CACHE[key] = build_nc()
    nc = _NC_CACHE[key]
    in_maps = make_in_maps(x, *args)
    res = run_bass_kernel_spmd(nc, in_maps, list(range(N_CORES)))
    return assemble_output(res.results)


if __name__ == "__main__":
    nc = build_nc(T=2)
    print("built ok")


# revision 17
# speedup vs baseline: 3.4493x; 3.4493x over previous
"""Trainium2 Bass kernel for nn_ActorNetSpiking (4-layer LIF SNN, T=50).

Contract: kernel(**inputs) takes FULL unsharded inputs (x:[4096,512,50] f32,
W1..W4/b1..b4, batch_size) and returns the FULL [4096,2] f32 output.

Strategy: pure data parallel over 8 NeuronCores (batch 4096 -> 512/core).
Host pre-transposes each core's x shard to [T, S, B]; activations live
transposed on-chip ([feature, batch]).

v2.3 design:
  * spikes s in {-1,+1} (no-spike = -1), produced by the Act engine:
    s' = Sign(ME2' - thr2) with per-neuron threshold as activation bias.
  * device states are shifted (steady-state/bias folding) and scaled by 2
    (exact in fp32):  SY2 = 2*(syn - syn_inf), ME2 = 2*(mem - mem_inf).
    Per layer (1..3):
       SY2' = alpha*SY2 + P            P = psum = Wdev @ s_in   (fp32r PE)
       u2   = beta*ME2 + SY2'          (STT, DVE)
       ME2' = u2 - s_own               (TensorTensor, Pool engine!)
       s'   = Sign(ME2' - thr2)        (Act)
    The x2 scaling turns the reset (-0.5*s) into an exact TT subtract that
    the Pool engine supports (walrus rejects STT on Pool).
  * layer 4 runs TRANSPOSED ([batch 128-part, (bchunk, action) free=8]) so
    its elementwise ops are ~60x cheaper; spikes r4 in {0,1} via TT is_gt
    with a broadcast threshold tile; acc += r4.
  * all matmul operands are native fp32r (1 cyc/row); states stay fp32.
"""

import sys

sys.path.insert(0, "/opt/trn_rl_repo")

from contextlib import ExitStack

import numpy as np

import concourse.bass as bass
import concourse.bacc as bacc
import concourse.tile as tile
from concourse import mybir

F32 = mybir.dt.float32
F32R = mybir.dt.float32r
ALU = mybir.AluOpType
ACT = mybir.ActivationFunctionType

ALPHA = 0.9
BETA = 0.85
THR = 1.0

N_CORES = 8
B_FULL = 4096
S = 512
H = 256
A = 2
T_FULL = 50
B = B_FULL // N_CORES  # 512 per core
BC = B // 128  # 4 batch chunks for transposed layer 4

# engine for each op kind (walrus: STT only on DVE; TT also on Pool)
DEFAULT_ASSIGN = {
    "syn": "vector",  # STT, psum src
    "u": "vector",  # STT
    "mem": "gpsimd",  # TT subtract
}


def build_nc(T=T_FULL, mm_f32r=True, assign=None):
    """Build the single-core Bass program (SPMD: same program on all cores)."""
    asn = dict(DEFAULT_ASSIGN)
    asn.update(assign or {})
    nc = bacc.Bacc(None, target_bir_lowering=False)

    MMDT = F32R if mm_f32r else F32
    xt = nc.declare_dram_parameter("xt", [T, S, B], MMDT, isOutput=False)
    w1t = nc.declare_dram_parameter("w1t", [S, H], MMDT, isOutput=False)
    w2t = nc.declare_dram_parameter("w2t", [H, H], MMDT, isOutput=False)
    w3t = nc.declare_dram_parameter("w3t", [H, H], MMDT, isOutput=False)
    w4t = nc.declare_dram_parameter("w4t", [H, A], MMDT, isOutput=False)
    nthr = {
        L: nc.declare_dram_parameter(f"nthr{L}", [H, 1], F32, isOutput=False)
        for L in (1, 2, 3)
    }
    thr4 = nc.declare_dram_parameter("thr4", [128, 2 * BC], F32, isOutput=False)
    isyn = {
        L: nc.declare_dram_parameter(f"isyn{L}", [H, B], F32, isOutput=False)
        for L in (1, 2, 3)
    }
    imem = {
        L: nc.declare_dram_parameter(f"imem{L}", [H, B], F32, isOutput=False)
        for L in (1, 2, 3)
    }
    isyn4 = nc.declare_dram_parameter("isyn4", [128, 2 * BC], F32, isOutput=False)
    imem4 = nc.declare_dram_parameter("imem4", [128, 2 * BC], F32, isOutput=False)
    sinit = nc.declare_dram_parameter("sinit", [128, B], MMDT, isOutput=False)
    out = nc.declare_dram_parameter("out", [128, 2 * BC], F32, isOutput=True)

    KC1 = S // 128  # 4 k-chunks for layer 1
    KC = H // 128  # 2 k-chunks for layers 2-4
    MC = H // 128  # 2 m-chunks for layers 1-3

    def eng(kind):
        return getattr(nc, asn[kind])

    with tile.TileContext(nc) as tc, ExitStack() as ctx:
        wp = ctx.enter_context(tc.tile_pool(name="weights", bufs=1))
        xp = ctx.enter_context(tc.tile_pool(name="x", bufs=3))
        sp = ctx.enter_context(tc.tile_pool(name="state", bufs=2))
        tp = ctx.enter_context(tc.tile_pool(name="tmp", bufs=2))
        pp = ctx.enter_context(tc.tile_pool(name="psum", bufs=1, space="PSUM"))

        # --- load weights ---
        w1 = []
        for k in range(KC1):
            wt = wp.tile([128, H], MMDT, tag=f"w1_{k}")
            nc.sync.dma_start(wt[:], w1t[k * 128 : (k + 1) * 128, :])
            w1.append(wt)
        w23 = {}
        for name, dram in (("w2", w2t), ("w3", w3t)):
            lst = []
            for k in range(KC):
                wt = wp.tile([128, H], MMDT, tag=f"{name}_{k}")
                nc.sync.dma_start(wt[:], dram[k * 128 : (k + 1) * 128, :])
                lst.append(wt)
            w23[name] = lst
        w4 = []
        for k in range(KC):
            wt = wp.tile([128, A], MMDT, tag=f"w4_{k}")
            nc.sync.dma_start(wt[:], w4t[k * 128 : (k + 1) * 128, :])
            w4.append(wt)

        # thresholds: negated [H,1] per chunk for Act bias; [128, 8] for L4
        nthr_t = {}
        for L in (1, 2, 3):
            for m in range(MC):
                t_ = wp.tile([128, 1], F32, tag=f"nthr{L}_{m}")
                nc.sync.dma_start(t_[:], nthr[L][m * 128 : (m + 1) * 128, :])
                nthr_t[(L, m)] = t_
        thr4_t = wp.tile([128, 2 * BC], F32, tag="thr4")
        nc.sync.dma_start(thr4_t[:], thr4[:])

        # --- initial states ---
        state = {}
        for L in (1, 2, 3):
            for m in range(MC):
                st = sp.tile([128, B], F32, tag=f"sy{L}_{m}")
                nc.sync.dma_start(st[:], isyn[L][m * 128 : (m + 1) * 128, :])
                state[("sy", L, m)] = st
                mt = sp.tile([128, B], F32, tag=f"me{L}_{m}")
                nc.sync.dma_start(mt[:], imem[L][m * 128 : (m + 1) * 128, :])
                state[("me", L, m)] = mt
                pt = sp.tile([128, B], MMDT, tag=f"s{L}_{m}")
                nc.sync.dma_start(pt[:], sinit[:, :])  # no-spike == -1
                state[("s", L, m)] = pt
        sy4 = sp.tile([128, 2 * BC], F32, tag="sy4")
        nc.sync.dma_start(sy4[:], isyn4[:])
        me4 = sp.tile([128, 2 * BC], F32, tag="me4")
        nc.sync.dma_start(me4[:], imem4[:])
        r4 = sp.tile([128, 2 * BC], F32, tag="r4")
        nc.vector.memset(r4[:], 0.0)
        acc = sp.tile([128, 2 * BC], F32, tag="acc")
        nc.vector.memset(acc[:], 0.0)
        state[("sy", 4)] = sy4
        state[("me", 4)] = me4
        state[("r", 4)] = r4

        def lif_update(L, m, cur_psum):
            """Layers 1-3, x2-scaled, +-1 spikes."""
            sy_o = state[("sy", L, m)]
            me_o = state[("me", L, m)]
            s_o = state[("s", L, m)]
            sy_n = sp.tile([128, B], F32, tag=f"sy{L}_{m}")
            eng("syn").scalar_tensor_tensor(
                sy_n[:], sy_o[:], ALPHA, cur_psum[:], op0=ALU.mult, op1=ALU.add
            )
            u = tp.tile([128, B], F32, tag=f"u{L}_{m}")
            eng("u").scalar_tensor_tensor(
                u[:], me_o[:], BETA, sy_n[:], op0=ALU.mult, op1=ALU.add
            )
            me_n = sp.tile([128, B], F32, tag=f"me{L}_{m}")
            eng("mem").tensor_tensor(
                me_n[:], u[:], s_o[:].bitcast(F32), op=ALU.subtract
            )
            s_n = sp.tile([128, B], MMDT, tag=f"s{L}_{m}")
            nc.scalar.activation(
                s_n[:], me_n[:], ACT.Sign, bias=nthr_t[(L, m)][:], scale=1.0
            )
            state[("sy", L, m)] = sy_n
            state[("me", L, m)] = me_n
            state[("s", L, m)] = s_n

        # spike-tile history per layer (read by the next layer one iteration
        # later under the skewed pipeline; pool bufs=2 covers the lifetime)
        shist = {1: [], 2: [], 3: []}

        def emit_l1(t):
            xtiles = []
            for k in range(KC1):
                xt_k = xp.tile([128, B], MMDT, tag=f"x_{k}")
                nc.sync.dma_start(xt_k[:], xt[t, k * 128 : (k + 1) * 128, :])
                xtiles.append(xt_k)
            for m in range(MC):
                ps = pp.tile([128, B], F32, tag=f"ps1_{m}")
                for k in range(KC1):
                    nc.tensor.matmul(
                        ps[:],
                        w1[k][:, m * 128 : (m + 1) * 128],
                        xtiles[k][:],
                        start=(k == 0),
                        stop=(k == KC1 - 1),
                    )
                lif_update(1, m, ps)
            shist[1].append((state[("s", 1, 0)], state[("s", 1, 1)]))

        def emit_l23(L, wname, t):
            sin = shist[L - 1][t]
            for m in range(MC):
                ps = pp.tile([128, B], F32, tag=f"ps{L}_{m}")
                for k in range(KC):
                    nc.tensor.matmul(
                        ps[:],
                        w23[wname][k][:, m * 128 : (m + 1) * 128],
                        sin[k][:],
                        start=(k == 0),
                        stop=(k == KC - 1),
                    )
                lif_update(L, m, ps)
            shist[L].append((state[("s", L, 0)], state[("s", L, 1)]))

        def emit_l4(t):
            nonlocal acc
            sin = shist[3][t]
            ps4 = pp.tile([128, 2 * BC], F32, tag="ps4")
            for c in range(BC):
                for k in range(KC):
                    nc.tensor.matmul(
                        ps4[:, 2 * c : 2 * c + 2],
                        sin[k][:, c * 128 : (c + 1) * 128],
                        w4[k][:],
                        start=(k == 0),
                        stop=(k == KC - 1),
                    )
            sy4_o, me4_o, r4_o = state[("sy", 4)], state[("me", 4)], state[("r", 4)]
            sy4_n = sp.tile([128, 2 * BC], F32, tag="sy4")
            nc.vector.scalar_tensor_tensor(
                sy4_n[:], sy4_o[:], ALPHA, ps4[:], op0=ALU.mult, op1=ALU.add
            )
            u4 = tp.tile([128, 2 * BC], F32, tag="u4")
            nc.vector.scalar_tensor_tensor(
                u4[:], me4_o[:], BETA, sy4_n[:], op0=ALU.mult, op1=ALU.add
            )
            me4_n = sp.tile([128, 2 * BC], F32, tag="me4")
            nc.gpsimd.tensor_tensor(me4_n[:], u4[:], r4_o[:], op=ALU.subtract)
            r4_n = sp.tile([128, 2 * BC], F32, tag="r4")
            nc.gpsimd.tensor_tensor(r4_n[:], me4_n[:], thr4_t[:], op=ALU.is_gt)
            acc_n = sp.tile([128, 2 * BC], F32, tag="acc")
            nc.gpsimd.tensor_tensor(acc_n[:], acc[:], r4_n[:], op=ALU.add)
            state[("sy", 4)] = sy4_n
            state[("me", 4)] = me4_n
            state[("r", 4)] = r4_n
            acc = acc_n

        # skewed pipeline: iteration i runs L1@t=i, L2@t=i-1, L3@t=i-2,
        # L4@t=i-3 -- every cross-layer input comes from a prior iteration,
        # so the four layer chains schedule independently.
        for i in range(T + 3):
            if i < T:
                emit_l1(i)
            if 0 <= i - 1 < T:
                emit_l23(2, "w2", i - 1)
            if 0 <= i - 2 < T:
                emit_l23(3, "w3", i - 2)
            if 0 <= i - 3 < T:
                emit_l4(i - 3)

        res = tp.tile([128, 2 * BC], F32, tag="res")
        nc.vector.tensor_copy(res[:], acc[:])
        nc.sync.dma_start(out[:], res[:])

    nc.finalize()
    return nc


def fold_params(W1, b1, W2, b2, W3, b3, W4, b4):
    """Host-side folding: +-1 spikes, steady-state shifts, x2 scaling.

    Device weights: w1 = 2*W1; w2 = W2; w3 = W3 (x2 scale cancels the /2 of
    the +-1 encoding); w4 = W4/2 (layer 4 unscaled states).
    """
    f8 = np.float64
    out = {}
    Ws = {1: W1.astype(f8), 2: W2.astype(f8), 3: W3.astype(f8), 4: W4.astype(f8)}
    bs = {1: b1.astype(f8), 2: b2.astype(f8), 3: b3.astype(f8), 4: b4.astype(f8)}
    # +-1 encoding for spike inputs of layers 2..4
    beff = {1: bs[1]}
    for L in (2, 3, 4):
        beff[L] = bs[L] + 0.5 * Ws[L].sum(axis=1)
    # device weights
    out["w1t"] = np.ascontiguousarray((2.0 * Ws[1]).T.astype(np.float32))
    out["w2t"] = np.ascontiguousarray(Ws[2].T.astype(np.float32))
    out["w3t"] = np.ascontiguousarray(Ws[3].T.astype(np.float32))
    out["w4t"] = np.ascontiguousarray((0.5 * Ws[4]).T.astype(np.float32))

    # layers 1-3: x2-scaled shifted states, +-1 own-spike reset (-0.5s - 0.5)
    for L in (1, 2, 3):
        syn_inf = beff[L] / (1.0 - ALPHA)
        mem_inf = (syn_inf - 0.5) / (1.0 - BETA)
        thr2 = 2.0 * (THR - mem_inf)
        out[f"nthr{L}"] = (-thr2[:, None]).astype(np.float32)
        out[f"isyn{L}"] = np.ascontiguousarray(
            np.broadcast_to((-2.0 * syn_inf[:, None]).astype(np.float32), (H, B))
        )
        out[f"imem{L}"] = np.ascontiguousarray(
            np.broadcast_to((-2.0 * mem_inf[:, None]).astype(np.float32), (H, B))
        )
    # layer 4: unscaled, 0/1 reset
    syn_inf4 = beff[4] / (1.0 - ALPHA)  # [A]
    mem_inf4 = syn_inf4 / (1.0 - BETA)
    thr4 = THR - mem_inf4  # [A]
    # transposed layout [128, (bchunk, action)]
    out["thr4"] = np.ascontiguousarray(
        np.broadcast_to(
            np.tile(thr4, BC)[None, :].astype(np.float32), (128, 2 * BC)
        )
    )
    out["isyn4"] = np.ascontiguousarray(
        np.broadcast_to(
            np.tile(-syn_inf4, BC)[None, :].astype(np.float32), (128, 2 * BC)
        )
    )
    out["imem4"] = np.ascontiguousarray(
        np.broadcast_to(
            np.tile(-mem_inf4, BC)[None, :].astype(np.float32), (128, 2 * BC)
        )
    )
    out["sinit"] = np.full((128, B), -1.0, np.float32)
    return out


def make_in_maps(x, W1, b1, W2, b2, W3, b3, W4, b4, T=T_FULL):
    """Shard + transpose full inputs into per-core input maps."""
    common = fold_params(W1, b1, W2, b2, W3, b3, W4, b4)
    in_maps = []
    for c in range(N_CORES):
        xs = x[c * B : (c + 1) * B, :, :T]  # [B, S, T]
        xtc = np.ascontiguousarray(xs.transpose(2, 1, 0))  # [T, S, B]
        m = dict(common)
        m["xt"] = xtc
        in_maps.append(m)
    return in_maps


def assemble_output(results, T=T_FULL):
    """results: per-core dicts with 'out' [128, 2*BC] raw spike counts."""
    outs = []
    for c in range(N_CORES):
        acc = results[c]["out"]  # [128, (bchunk, action)]
        per = acc.reshape(128, BC, A).transpose(1, 0, 2).reshape(B, A)
        outs.append(per)
    full = np.concatenate(outs, axis=0)  # [4096, A]
    return (full / np.float32(T)).astype(np.float32)


_NC_CACHE = {}


def kernel(x, W1, b1, W2, b2, W3, b3, W4, b4, batch_size=None, **_):
    x = np.asarray(x, np.float32)
    args = [np.asarray(a, np.float32) for a in (W1, b1, W2, b2, W3, b3, W4, b4)]
    from concourse.bass_utils import run_bass_kernel_spmd

    key = "main"
    if key not in _NC_CACHE:
        _NC_CACHE[key] = build_nc()
    nc = _NC_CACHE[key]
    in_maps = make_in_maps(x, *args)
    res = run_bass_kernel_spmd(nc, in_maps, list(range(N_CORES)))
    return assemble_output(res.results)


if __name__ == "__main__":
    nc = build_nc(T=2)
    print("built ok")


# revision 22
# speedup vs baseline: 53.5920x; 15.5370x over previous
"""Trainium2 Bass kernel for nn_ActorNetSpiking (4-layer LIF SNN, T=50).

Contract: kernel(**inputs) takes FULL unsharded inputs (x:[4096,512,50] f32,
W1..W4/b1..b4, batch_size) and returns the FULL [4096,2] f32 output.

Strategy: pure data parallel over 8 NeuronCores (batch 4096 -> 512/core).
Host pre-transposes each core's x shard to [T, S, B]; activations live
transposed on-chip ([feature, batch]).

v2.3 design:
  * spikes s in {-1,+1} (no-spike = -1), produced by the Act engine:
    s' = Sign(ME2' - thr2) with per-neuron threshold as activation bias.
  * device states are shifted (steady-state/bias folding) and scaled by 2
    (exact in fp32):  SY2 = 2*(syn - syn_inf), ME2 = 2*(mem - mem_inf).
    Per layer (1..3):
       SY2' = alpha*SY2 + P            P = psum = Wdev @ s_in   (fp32r PE)
       u2   = beta*ME2 + SY2'          (STT, DVE)
       ME2' = u2 - s_own               (TensorTensor, Pool engine!)
       s'   = Sign(ME2' - thr2)        (Act)
    The x2 scaling turns the reset (-0.5*s) into an exact TT subtract that
    the Pool engine supports (walrus rejects STT on Pool).
  * layer 4 runs TRANSPOSED ([batch 128-part, (bchunk, action) free=8]) so
    its elementwise ops are ~60x cheaper; spikes r4 in {0,1} via TT is_gt
    with a broadcast threshold tile; acc += r4.
  * all matmul operands are native fp32r (1 cyc/row); states stay fp32.
"""

import sys

sys.path.insert(0, "/opt/trn_rl_repo")

from contextlib import ExitStack

import numpy as np

import concourse.bass as bass
import concourse.bacc as bacc
import concourse.tile as tile
from concourse import mybir

F32 = mybir.dt.float32
F32R = mybir.dt.float32r
ALU = mybir.AluOpType
ACT = mybir.ActivationFunctionType

ALPHA = 0.9
BETA = 0.85
THR = 1.0

N_CORES = 8
B_FULL = 4096
S = 512
H = 256
A = 2
T_FULL = 50
B = B_FULL // N_CORES  # 512 per core
BC = B // 128  # 4 batch chunks for transposed layer 4

# engine for each op kind (walrus: STT only on DVE; TT also on Pool)
DEFAULT_ASSIGN = {
    "syn": "vector",  # STT, psum src
    "u": "vector",  # STT
    "mem": "gpsimd",  # TT subtract
}


def build_nc(T=T_FULL, mm_f32r=True, assign=None, reps=1, dummy_x=False):
    """Build the single-core Bass program (SPMD: same program on all cores).

    dummy_x=True replaces the x input with an internal (uninitialized) DRAM
    tensor — identical DMA/compute structure without the 419MB host upload;
    used only for timing builds.
    """
    asn = dict(DEFAULT_ASSIGN)
    asn.update(assign or {})
    nc = bacc.Bacc(None, target_bir_lowering=False)

    MMDT = F32R if mm_f32r else F32
    if dummy_x:
        xt = nc.dram_tensor("xt_dummy", [T, S, B], MMDT)
    else:
        xt = nc.declare_dram_parameter("xt", [T, S, B], MMDT, isOutput=False)
    w1t = nc.declare_dram_parameter("w1t", [S, H], MMDT, isOutput=False)
    w2t = nc.declare_dram_parameter("w2t", [H, H], MMDT, isOutput=False)
    w3t = nc.declare_dram_parameter("w3t", [H, H], MMDT, isOutput=False)
    w4t = nc.declare_dram_parameter("w4t", [H, A], MMDT, isOutput=False)
    nthr = {
        L: nc.declare_dram_parameter(f"nthr{L}", [H, 1], F32, isOutput=False)
        for L in (1, 2, 3)
    }
    thr4 = nc.declare_dram_parameter("thr4", [128, 2 * BC], F32, isOutput=False)
    isyn = {
        L: nc.declare_dram_parameter(f"isyn{L}", [H, B], F32, isOutput=False)
        for L in (1, 2, 3)
    }
    imem = {
        L: nc.declare_dram_parameter(f"imem{L}", [H, B], F32, isOutput=False)
        for L in (1, 2, 3)
    }
    isyn4 = nc.declare_dram_parameter("isyn4", [128, 2 * BC], F32, isOutput=False)
    imem4 = nc.declare_dram_parameter("imem4", [128, 2 * BC], F32, isOutput=False)
    sinit = nc.declare_dram_parameter("sinit", [128, B], MMDT, isOutput=False)
    out = nc.declare_dram_parameter("out", [128, 2 * BC], F32, isOutput=True)

    KC1 = S // 128  # 4 k-chunks for layer 1
    KC = H // 128  # 2 k-chunks for layers 2-4
    MC = H // 128  # 2 m-chunks for layers 1-3

    def eng(kind):
        return getattr(nc, asn[kind])

    with tile.TileContext(nc) as tc, ExitStack() as ctx:
        wp = ctx.enter_context(tc.tile_pool(name="weights", bufs=1))
        xp = ctx.enter_context(tc.tile_pool(name="x", bufs=3))
        sp = ctx.enter_context(tc.tile_pool(name="state", bufs=2))
        tp = ctx.enter_context(tc.tile_pool(name="tmp", bufs=2))
        pp = ctx.enter_context(tc.tile_pool(name="psum", bufs=1, space="PSUM"))

        # --- load weights ---
        w1 = []
        for k in range(KC1):
            wt = wp.tile([128, H], MMDT, tag=f"w1_{k}")
            nc.sync.dma_start(wt[:], w1t[k * 128 : (k + 1) * 128, :])
            w1.append(wt)
        w23 = {}
        for name, dram in (("w2", w2t), ("w3", w3t)):
            lst = []
            for k in range(KC):
                wt = wp.tile([128, H], MMDT, tag=f"{name}_{k}")
                nc.sync.dma_start(wt[:], dram[k * 128 : (k + 1) * 128, :])
                lst.append(wt)
            w23[name] = lst
        w4 = []
        for k in range(KC):
            wt = wp.tile([128, A], MMDT, tag=f"w4_{k}")
            nc.sync.dma_start(wt[:], w4t[k * 128 : (k + 1) * 128, :])
            w4.append(wt)

        # thresholds: negated [H,1] per chunk for Act bias; [128, 8] for L4
        nthr_t = {}
        for L in (1, 2, 3):
            for m in range(MC):
                t_ = wp.tile([128, 1], F32, tag=f"nthr{L}_{m}")
                nc.sync.dma_start(t_[:], nthr[L][m * 128 : (m + 1) * 128, :])
                nthr_t[(L, m)] = t_
        thr4_t = wp.tile([128, 2 * BC], F32, tag="thr4")
        nc.sync.dma_start(thr4_t[:], thr4[:])

        # --- initial states ---
        state = {}
        for L in (1, 2, 3):
            for m in range(MC):
                st = sp.tile([128, B], F32, tag=f"sy{L}_{m}")
                nc.sync.dma_start(st[:], isyn[L][m * 128 : (m + 1) * 128, :])
                state[("sy", L, m)] = st
                mt = sp.tile([128, B], F32, tag=f"me{L}_{m}")
                nc.sync.dma_start(mt[:], imem[L][m * 128 : (m + 1) * 128, :])
                state[("me", L, m)] = mt
                pt = sp.tile([128, B], MMDT, tag=f"s{L}_{m}")
                nc.sync.dma_start(pt[:], sinit[:, :])  # no-spike == -1
                state[("s", L, m)] = pt
        sy4 = sp.tile([128, 2 * BC], F32, tag="sy4")
        nc.sync.dma_start(sy4[:], isyn4[:])
        me4 = sp.tile([128, 2 * BC], F32, tag="me4")
        nc.sync.dma_start(me4[:], imem4[:])
        r4 = sp.tile([128, 2 * BC], F32, tag="r4")
        nc.vector.memset(r4[:], 0.0)
        acc = sp.tile([128, 2 * BC], F32, tag="acc")
        nc.vector.memset(acc[:], 0.0)
        state[("sy", 4)] = sy4
        state[("me", 4)] = me4
        state[("r", 4)] = r4

        def lif_update(L, m, cur_psum):
            """Layers 1-3, x2-scaled, +-1 spikes."""
            sy_o = state[("sy", L, m)]
            me_o = state[("me", L, m)]
            s_o = state[("s", L, m)]
            sy_n = sp.tile([128, B], F32, tag=f"sy{L}_{m}")
            eng("syn").scalar_tensor_tensor(
                sy_n[:], sy_o[:], ALPHA, cur_psum[:], op0=ALU.mult, op1=ALU.add
            )
            u = tp.tile([128, B], F32, tag=f"u{L}_{m}")
            eng("u").scalar_tensor_tensor(
                u[:], me_o[:], BETA, sy_n[:], op0=ALU.mult, op1=ALU.add
            )
            me_n = sp.tile([128, B], F32, tag=f"me{L}_{m}")
            eng("mem").tensor_tensor(
                me_n[:], u[:], s_o[:].bitcast(F32), op=ALU.subtract
            )
            s_n = sp.tile([128, B], MMDT, tag=f"s{L}_{m}")
            nc.scalar.activation(
                s_n[:], me_n[:], ACT.Sign, bias=nthr_t[(L, m)][:], scale=1.0
            )
            state[("sy", L, m)] = sy_n
            state[("me", L, m)] = me_n
            state[("s", L, m)] = s_n

        # spike-tile history per layer (read by the next layer one iteration
        # later under the skewed pipeline; pool bufs=2 covers the lifetime)
        shist = {1: [], 2: [], 3: []}

        def emit_l1(t):
            xtiles = []
            for k in range(KC1):
                xt_k = xp.tile([128, B], MMDT, tag=f"x_{k}")
                nc.sync.dma_start(xt_k[:], xt[t, k * 128 : (k + 1) * 128, :])
                xtiles.append(xt_k)
            for m in range(MC):
                ps = pp.tile([128, B], F32, tag=f"ps1_{m}")
                for k in range(KC1):
                    nc.tensor.matmul(
                        ps[:],
                        w1[k][:, m * 128 : (m + 1) * 128],
                        xtiles[k][:],
                        start=(k == 0),
                        stop=(k == KC1 - 1),
                    )
                lif_update(1, m, ps)
            shist[1].append((state[("s", 1, 0)], state[("s", 1, 1)]))

        def emit_l23(L, wname, t):
            sin = shist[L - 1][t]
            for m in range(MC):
                ps = pp.tile([128, B], F32, tag=f"ps{L}_{m}")
                for k in range(KC):
                    nc.tensor.matmul(
                        ps[:],
                        w23[wname][k][:, m * 128 : (m + 1) * 128],
                        sin[k][:],
                        start=(k == 0),
                        stop=(k == KC - 1),
                    )
                lif_update(L, m, ps)
            shist[L].append((state[("s", L, 0)], state[("s", L, 1)]))

        def emit_l4(t):
            nonlocal acc
            sin = shist[3][t]
            ps4 = pp.tile([128, 2 * BC], F32, tag="ps4")
            for c in range(BC):
                for k in range(KC):
                    nc.tensor.matmul(
                        ps4[:, 2 * c : 2 * c + 2],
                        sin[k][:, c * 128 : (c + 1) * 128],
                        w4[k][:],
                        start=(k == 0),
                        stop=(k == KC - 1),
                    )
            sy4_o, me4_o, r4_o = state[("sy", 4)], state[("me", 4)], state[("r", 4)]
            sy4_n = sp.tile([128, 2 * BC], F32, tag="sy4")
            nc.vector.scalar_tensor_tensor(
                sy4_n[:], sy4_o[:], ALPHA, ps4[:], op0=ALU.mult, op1=ALU.add
            )
            u4 = tp.tile([128, 2 * BC], F32, tag="u4")
            nc.vector.scalar_tensor_tensor(
                u4[:], me4_o[:], BETA, sy4_n[:], op0=ALU.mult, op1=ALU.add
            )
            me4_n = sp.tile([128, 2 * BC], F32, tag="me4")
            nc.vector.tensor_tensor(me4_n[:], u4[:], r4_o[:], op=ALU.subtract)
            r4_n = sp.tile([128, 2 * BC], F32, tag="r4")
            nc.vector.tensor_tensor(r4_n[:], me4_n[:], thr4_t[:], op=ALU.is_gt)
            acc_n = sp.tile([128, 2 * BC], F32, tag="acc")
            nc.vector.tensor_tensor(acc_n[:], acc[:], r4_n[:], op=ALU.add)
            state[("sy", 4)] = sy4_n
            state[("me", 4)] = me4_n
            state[("r", 4)] = r4_n
            acc = acc_n

        # skewed pipeline: iteration i runs L1@t=i, L2@t=i-1, L3@t=i-2,
        # L4@t=i-3 -- every cross-layer input comes from a prior iteration,
        # so the four layer chains schedule independently.
        # reps>1 re-runs the whole dynamics for in-NEFF timing builds only.
        for _rep in range(reps):
            for lst in shist.values():
                lst.clear()
            for i in range(T + 3):
                if i < T:
                    emit_l1(i)
                if 0 <= i - 1 < T:
                    emit_l23(2, "w2", i - 1)
                if 0 <= i - 2 < T:
                    emit_l23(3, "w3", i - 2)
                if 0 <= i - 3 < T:
                    emit_l4(i - 3)

        res = tp.tile([128, 2 * BC], F32, tag="res")
        nc.vector.tensor_copy(res[:], acc[:])
        nc.sync.dma_start(out[:], res[:])

    nc.finalize()
    return nc


def fold_params(W1, b1, W2, b2, W3, b3, W4, b4):
    """Host-side folding: +-1 spikes, steady-state shifts, x2 scaling.

    Device weights: w1 = 2*W1; w2 = W2; w3 = W3 (x2 scale cancels the /2 of
    the +-1 encoding); w4 = W4/2 (layer 4 unscaled states).
    """
    f8 = np.float64
    out = {}
    Ws = {1: W1.astype(f8), 2: W2.astype(f8), 3: W3.astype(f8), 4: W4.astype(f8)}
    bs = {1: b1.astype(f8), 2: b2.astype(f8), 3: b3.astype(f8), 4: b4.astype(f8)}
    # +-1 encoding for spike inputs of layers 2..4
    beff = {1: bs[1]}
    for L in (2, 3, 4):
        beff[L] = bs[L] + 0.5 * Ws[L].sum(axis=1)
    # device weights
    out["w1t"] = np.ascontiguousarray((2.0 * Ws[1]).T.astype(np.float32))
    out["w2t"] = np.ascontiguousarray(Ws[2].T.astype(np.float32))
    out["w3t"] = np.ascontiguousarray(Ws[3].T.astype(np.float32))
    out["w4t"] = np.ascontiguousarray((0.5 * Ws[4]).T.astype(np.float32))

    # layers 1-3: x2-scaled shifted states, +-1 own-spike reset (-0.5s - 0.5)
    for L in (1, 2, 3):
        syn_inf = beff[L] / (1.0 - ALPHA)
        mem_inf = (syn_inf - 0.5) / (1.0 - BETA)
        thr2 = 2.0 * (THR - mem_inf)
        out[f"nthr{L}"] = (-thr2[:, None]).astype(np.float32)
        out[f"isyn{L}"] = np.ascontiguousarray(
            np.broadcast_to((-2.0 * syn_inf[:, None]).astype(np.float32), (H, B))
        )
        out[f"imem{L}"] = np.ascontiguousarray(
            np.broadcast_to((-2.0 * mem_inf[:, None]).astype(np.float32), (H, B))
        )
    # layer 4: unscaled, 0/1 reset
    syn_inf4 = beff[4] / (1.0 - ALPHA)  # [A]
    mem_inf4 = syn_inf4 / (1.0 - BETA)
    thr4 = THR - mem_inf4  # [A]
    # transposed layout [128, (bchunk, action)]
    out["thr4"] = np.ascontiguousarray(
        np.broadcast_to(
            np.tile(thr4, BC)[None, :].astype(np.float32), (128, 2 * BC)
        )
    )
    out["isyn4"] = np.ascontiguousarray(
        np.broadcast_to(
            np.tile(-syn_inf4, BC)[None, :].astype(np.float32), (128, 2 * BC)
        )
    )
    out["imem4"] = np.ascontiguousarray(
        np.broadcast_to(
            np.tile(-mem_inf4, BC)[None, :].astype(np.float32), (128, 2 * BC)
        )
    )
    out["sinit"] = np.full((128, B), -1.0, np.float32)
    return out


def make_in_maps(x, W1, b1, W2, b2, W3, b3, W4, b4, T=T_FULL):
    """Shard + transpose full inputs into per-core input maps."""
    common = fold_params(W1, b1, W2, b2, W3, b3, W4, b4)
    in_maps = []
    for c in range(N_CORES):
        xs = x[c * B : (c + 1) * B, :, :T]  # [B, S, T]
        xtc = np.ascontiguousarray(xs.transpose(2, 1, 0))  # [T, S, B]
        m = dict(common)
        m["xt"] = xtc
        in_maps.append(m)
    return in_maps


def assemble_output(results, T=T_FULL):
    """results: per-core dicts with 'out' [128, 2*BC] raw spike counts."""
    outs = []
    for c in range(N_CORES):
        acc = results[c]["out"]  # [128, (bchunk, action)]
        per = acc.reshape(128, BC, A).transpose(1, 0, 2).reshape(B, A)
        outs.append(per)
    full = np.concatenate(outs, axis=0)  # [4096, A]
    return (full / np.float32(T)).astype(np.float32)


_NC_CACHE = {}


def kernel(x, W1, b1, W2, b2, W3, b3, W4, b4, batch_size=None, **_):
    x = np.asarray(x, np.float32)
    args = [np.asarray(a, np.float32) for a in (W1, b1, W2, b2, W3, b3, W4, b4)]
    from concourse.bass_utils import run_bass_kernel_spmd

    key = "main"
    if key not in _NC_CACHE:
        _NC_CACHE[key] = build_nc()
    nc = _NC_CACHE[key]
    in_maps = make_in_maps(x, *args)
    res = run_bass_kernel_spmd(nc, in_maps, list(range(N_CORES)))
    return assemble_output(res.results)


if __name__ == "__main__":
    nc = build_nc(T=2)
    print("built ok")
